# Initial kernel scaffold
#
"""Trainium2 Bass kernel for nn_DisentangledAttention (DeBERTa-style).

Strategy (batch-parallel over 8 cores, one batch element per core):
  - Only 2S-1 = 767 distinct relative positions exist, so the position
    projections run on the 767-row table instead of the [S,S,E] gather
    (cuts ~77G MACs to ~0.4G).
  - content-position (bd) and position-content (bc) scores are computed in
    "relative" coordinates [i, r] / [j, r'] by plain matmuls, then moved to
    absolute [i, j] coordinates with a skewed SBUF->SBUF DMA: a diagonal
    flat access pattern (step = row_stride - 1) gives each partition a
    free-dim offset of -1 per partition, which is exactly the relative->
    absolute shear.  bc lands transposed and is added into the score PSUM
    with PE transpose-accumulate matmuls.
  - softmax: ACT exp (scale=1/8 folded in) with fused row-sum accumulator,
    DVE reciprocal + per-partition scale.
  - attn @ V consumes w^T produced by PE transposes of the weight tiles;
    the result lands head-transposed [E, S] which feeds the final fc
    matmul directly (lhsT layout), so the fc output is row-major [S, E].
"""

import sys

sys.path.insert(0, "/opt/trn_rl_repo")

import numpy as np

S = 384
E = 512
H = 8
D = 64
P = 128
R = 2 * S - 1          # 767 distinct relative offsets
BAND = 511             # valid band width per 128-row tile
NE = E // P            # 4
NS = S // P            # 3
MAX_SEQ = 512
B = 8

_CACHE = {}


def _build():
    import concourse.bacc as bacc
    import concourse.mybir as mybir
    from concourse.tile import TileContext
    from concourse.masks import make_identity

    f32 = mybir.dt.float32

    nc = bacc.Bacc("TRN2", num_devices=B, debug=False)

    xT = nc.dram_tensor("xT", [E, S], f32, kind="ExternalInput")
    qW = nc.dram_tensor("qW", [E, E], f32, kind="ExternalInput")
    kW = nc.dram_tensor("kW", [E, E], f32, kind="ExternalInput")
    vW = nc.dram_tensor("vW", [E, E], f32, kind="ExternalInput")
    qpW = nc.dram_tensor("qpW", [E, E], f32, kind="ExternalInput")
    kpW = nc.dram_tensor("kpW", [E, E], f32, kind="ExternalInput")
    fcW = nc.dram_tensor("fcW", [E, E], f32, kind="ExternalInput")
    posT = nc.dram_tensor("posT", [E, R], f32, kind="ExternalInput")
    bias4 = nc.dram_tensor("bias4", [P, 16], f32, kind="ExternalInput")
    bias2 = nc.dram_tensor("bias2", [2, E], f32, kind="ExternalInput")

    w_out = nc.dram_tensor("w_out", [H, S, S], f32, kind="ExternalOutput")
    out_b = nc.dram_tensor("out_b", [S, E], f32, kind="ExternalOutput")

    Exp = mybir.ActivationFunctionType.Exp
    Copy = mybir.ActivationFunctionType.Copy
    Ident = mybir.ActivationFunctionType.Identity

    def diag_ap(tile_ap, width, offset, count, length):
        """AP reading tile[p, offset + c - p] for c in [0, length): diagonal
        flat AP with step (width-1) across partitions."""
        a = tile_ap.copy()
        a.ap = mybir.VecI64Pair([(width - 1, count), (1, length)])
        a.offset = offset
        return a

    with TileContext(nc) as tc:
        with (
            tc.tile_pool(name="const") as cpool,
            tc.tile_pool(name="proj_out") as projp,
            tc.tile_pool(name="pos_out") as posp,
            tc.tile_pool(name="attn_t") as attnp,
        ):
            ident = cpool.tile([P, P], f32)
            make_identity(nc, ident)
            ones = cpool.tile([1, P], f32)
            nc.gpsimd.memset(ones[:, :], 1.0)
            b4 = cpool.tile([P, 16], f32)
            nc.sync.dma_start(b4[:, :], bias4[:, :])
            b2 = cpool.tile([2, E], f32)
            nc.sync.dma_start(b2[:, :], bias2[:, :])

            qT_sb = projp.tile([P, NE * S], f32)   # [e_out, (m, s)]
            kT_sb = projp.tile([P, NE * S], f32)
            v_sb = projp.tile([P, NS * E], f32)    # [s, (m, e)]
            kpT_sb = posp.tile([P, NE * R], f32)
            qpTr_sb = posp.tile([P, NE * R], f32)  # reversed along r
            attnT_sb = attnp.tile([P, NE * S], f32)

            # ---------------- phase 1: projections ----------------
            with (
                tc.tile_pool(name="wts") as wpool,
                tc.tile_pool(name="ps_pj", space="PSUM") as pspj,
            ):
                x_sb = wpool.tile([P, NE * S], f32, tag="x")
                nc.sync.dma_start(
                    x_sb[:, :].rearrange("p (k s) -> p k s", s=S),
                    xT.rearrange("(k p) s -> p k s", p=P),
                )
                posT_sb = wpool.tile([P, NE * R], f32, tag="pos")
                nc.sync.dma_start(
                    posT_sb[:, :].rearrange("p (k r) -> p k r", r=R),
                    posT.rearrange("(k p) r -> p k r", p=P),
                )
                w_tiles = {}
                for name, dram in [("qW", qW), ("kW", kW), ("vW", vW),
                                   ("qpW", qpW), ("kpW", kpW), ("fcW", fcW)]:
                    t = wpool.tile([P, NE * E], f32, tag=name)
                    nc.sync.dma_start(
                        t[:, :].rearrange("p (k e) -> p k e", e=E),
                        dram.rearrange("(k p) e -> p k e", p=P),
                    )
                    w_tiles[name] = t

                # qT, kT: [e_out, s] with per-partition bias
                for dst, wname, bcol in [(qT_sb, "qW", 0), (kT_sb, "kW", 4)]:
                    wsb = w_tiles[wname]
                    for m in range(NE):
                        ps = pspj.tile([P, S], f32, tag="pj")
                        for k in range(NE):
                            nc.tensor.matmul(
                                ps[:, :],
                                wsb[:, k * E + m * P : k * E + (m + 1) * P],
                                x_sb[:, k * S : (k + 1) * S],
                                start=(k == 0), stop=(k == NE - 1),
                            )
                        nc.vector.tensor_scalar_add(
                            dst[:, m * S : (m + 1) * S], ps[:, :],
                            b4[:, bcol + m : bcol + m + 1],
                        )

                # v: [s, e] natural; bias via K=1 matmul with ones
                for m in range(NS):
                    ps = pspj.tile([P, E], f32, tag="pj")
                    for k in range(NE):
                        nc.tensor.matmul(
                            ps[:, :],
                            x_sb[:, k * S + m * P : k * S + (m + 1) * P],
                            w_tiles["vW"][:, k * E : (k + 1) * E],
                            start=(k == 0), stop=False,
                        )
                    nc.tensor.matmul(
                        ps[:, :], ones[:, :], b2[0:1, :],
                        start=False, stop=True, skip_group_check=True,
                    )
                    nc.scalar.activation(v_sb[:, m * E : (m + 1) * E], ps[:, :], Copy)

                # kpT / qpT(reversed): [e_out, r] with per-partition bias
                for dst, wname, bcol, rev in [(kpT_sb, "kpW", 8, False),
                                              (qpTr_sb, "qpW", 12, True)]:
                    wsb = w_tiles[wname]
                    for m in range(NE):
                        for n0, wdt in [(0, 512), (512, 255)]:
                            ps = pspj.tile([P, E], f32, tag="pj")
                            for k in range(NE):
                                nc.tensor.matmul(
                                    ps[:, 0:wdt],
                                    wsb[:, k * E + m * P : k * E + (m + 1) * P],
                                    posT_sb[:, k * R + n0 : k * R + n0 + wdt],
                                    start=(k == 0), stop=(k == NE - 1),
                                )
                            if not rev:
                                o = dst[:, m * R + n0 : m * R + n0 + wdt]
                            else:
                                o = dst[:, :].copy()
                                o.ap = mybir.VecI64Pair([(NE * R, P), (-1, wdt)])
                                o.offset = m * R + (R - 1) - n0
                            nc.scalar.activation(
                                o, ps[:, 0:wdt], Ident,
                                bias=b4[:, bcol + m : bcol + m + 1],
                            )

            # ---------------- phase 2: attention per head ----------------
            with (
                tc.tile_pool(name="band") as bandp,
                tc.tile_pool(name="skew") as skewp,
                tc.tile_pool(name="soft") as softp,
                tc.tile_pool(name="wT") as wTp,
                tc.tile_pool(name="ps_rel", space="PSUM") as psrel,
                tc.tile_pool(name="ps_s", space="PSUM") as pss,
                tc.tile_pool(name="ps_wt", space="PSUM") as pswt,
                tc.tile_pool(name="ps_o", space="PSUM") as pso,
            ):
                for h in range(H):
                    hb = 64 * (h % 2)
                    mh = h // 2

                    def hsl(tile, base, wdt):
                        return tile[hb : hb + 64, base : base + wdt]

                    # relative-coordinate score matrices (banded)
                    bd_t = []
                    bcT_t = []
                    for t in range(NS):
                        r0 = 256 - P * t
                        # bd_rel[i, r] = q[i] . kp[r]
                        psb = psrel.tile([P, BAND], f32, tag="rel")
                        nc.tensor.matmul(
                            psb[:, :],
                            hsl(qT_sb, mh * S + t * P, P),
                            hsl(kpT_sb, mh * R + r0, BAND),
                        )
                        band = bandp.tile([P, 512], f32, tag="bdband")
                        nc.scalar.activation(band[:, 0:BAND], psb[:, :], Copy)
                        sk = skewp.tile([P, S], f32, tag="bd")
                        nc.gpsimd.dma_start(sk[:, :], diag_ap(band[:, :], 512, 127, P, S))
                        bd_t.append(sk)
                        # bcT_rel[j, r'] = k[j] . qp_rev[r']
                        psb2 = psrel.tile([P, BAND], f32, tag="rel")
                        nc.tensor.matmul(
                            psb2[:, :],
                            hsl(kT_sb, mh * S + t * P, P),
                            hsl(qpTr_sb, mh * R + r0, BAND),
                        )
                        band2 = bandp.tile([P, 512], f32, tag="bcband")
                        nc.vector.tensor_copy(band2[:, 0:BAND], psb2[:, :])
                        sk2 = skewp.tile([P, S], f32, tag="bcT")
                        nc.gpsimd.dma_start(sk2[:, :], diag_ap(band2[:, :], 512, 127, P, S))
                        bcT_t.append(sk2)

                    # scores, softmax, weight tiles
                    w_sb = softp.tile([P, NS * S], f32, tag="w")
                    ps_wt_t = [pswt.tile([P, S], f32, tag="wt") for _ in range(NS)]
                    for t in range(NS):
                        ps_sc = pss.tile([P, S], f32, tag="sc")
                        nc.tensor.matmul(
                            ps_sc[:, :],
                            hsl(qT_sb, mh * S + t * P, P),
                            hsl(kT_sb, mh * S, S),
                            start=True, stop=False,
                        )
                        for tj in range(NS):
                            nc.tensor.matmul(
                                ps_sc[:, tj * P : (tj + 1) * P],
                                bcT_t[tj][:, t * P : (t + 1) * P],
                                ident[:, :],
                                is_transpose=True,
                                start=False, stop=(tj == NS - 1),
                                skip_group_check=True,
                            )
                        s1 = softp.tile([P, S], f32, tag="s1")
                        nc.vector.tensor_add(s1[:, :], ps_sc[:, :], bd_t[t][:, :])
                        ssum = softp.tile([P, 1], f32, tag="ssum")
                        es = softp.tile([P, S], f32, tag="es")
                        nc.scalar.activation(es[:, :], s1[:, :], Exp,
                                             scale=0.125, accum_out=ssum[:, :])
                        srec = softp.tile([P, 1], f32, tag="srec")
                        nc.vector.reciprocal(srec[:, :], ssum[:, :])
                        nc.vector.tensor_scalar_mul(
                            w_sb[:, t * S : (t + 1) * S], es[:, :], srec[:, :])
                        for tj in range(NS):
                            nc.tensor.matmul(
                                ps_wt_t[tj][:, t * P : (t + 1) * P],
                                w_sb[:, t * S + tj * P : t * S + (tj + 1) * P],
                                ident[:, :],
                                is_transpose=True,
                                start=True, stop=True, skip_group_check=True,
                            )

                    # weights output (straight [i, j] write)
                    nc.scalar.dma_start(
                        w_out[h].rearrange("(t p) j -> t p j", p=P),
                        w_sb[:, :].rearrange("p (t j) -> t p j", j=S),
                    )

                    # attn @ V -> outT [d, i] accumulated over j tiles
                    ps_out = pso.tile([64, S], f32, tag="av")
                    for tj in range(NS):
                        wT_sb = wTp.tile([P, S], f32, tag="wT")
                        nc.scalar.activation(wT_sb[:, :], ps_wt_t[tj][:, :], Copy)
                        nc.tensor.matmul(
                            ps_out[:, :],
                            v_sb[:, tj * E + 64 * h : tj * E + 64 * h + 64],
                            wT_sb[:, :],
                            start=(tj == 0), stop=(tj == NS - 1),
                        )
                    nc.scalar.activation(
                        attnT_sb[hb : hb + 64, mh * S : (mh + 1) * S],
                        ps_out[:, :], Copy)

            # ---------------- phase 3: fc ----------------
            with (
                tc.tile_pool(name="fc_sb") as fcp,
                tc.tile_pool(name="ps_fc", space="PSUM") as psfc,
            ):
                fcW_sb = fcp.tile([P, NE * E], f32, tag="fcw")
                nc.sync.dma_start(
                    fcW_sb[:, :].rearrange("p (k e) -> p k e", e=E),
                    fcW.rearrange("(k p) e -> p k e", p=P),
                )
                for m in range(NS):
                    ps = psfc.tile([P, E], f32, tag="fc")
                    for k in range(NE):
                        nc.tensor.matmul(
                            ps[:, :],
                            attnT_sb[:, k * S + m * P : k * S + (m + 1) * P],
                            fcW_sb[:, k * E : (k + 1) * E],
                            start=(k == 0), stop=False,
                        )
                    nc.tensor.matmul(
                        ps[:, :], ones[:, :], b2[1:2, :],
                        start=False, stop=True, skip_group_check=True,
                    )
                    o_sb = fcp.tile([P, E], f32, tag="o")
                    nc.scalar.activation(o_sb[:, :], ps[:, :], Copy)
                    nc.scalar.dma_start(out_b[m * P : (m + 1) * P, :], o_sb[:, :])

    nc.compile()
    return nc


def kernel(x, qW, qb, kW, kb, vW, vb, qpW, qpb, kpW, kpb, pos_table, fcW, fcb):
    from concourse import bass_utils

    if "nc" not in _CACHE:
        _CACHE["nc"] = _build()
    nc = _CACHE["nc"]

    f = lambda a: np.ascontiguousarray(np.asarray(a), dtype=np.float32)
    x, pos_table = f(x), f(pos_table)
    posT = np.ascontiguousarray(pos_table[MAX_SEQ - S : MAX_SEQ - S + R].T)
    bias4 = np.concatenate(
        [f(b).reshape(NE, P).T for b in (qb, kb, kpb, qpb)], axis=1)
    bias2 = np.stack([f(vb), f(fcb)])
    base = {"qW": f(qW), "kW": f(kW), "vW": f(vW), "qpW": f(qpW),
            "kpW": f(kpW), "fcW": f(fcW), "posT": posT,
            "bias4": np.ascontiguousarray(bias4), "bias2": np.ascontiguousarray(bias2)}
    in_maps = [dict(base, xT=np.ascontiguousarray(x[b].T)) for b in range(B)]

    res = bass_utils.run_bass_kernel_spmd(nc, in_maps, core_ids=list(range(B)))
    out = np.stack([r["out_b"] for r in res.results])
    weights = np.stack([r["w_out"] for r in res.results])
    _CACHE["last_results"] = res
    return out, weights


# revision 13
# speedup vs baseline: 16579.4601x; 16579.4601x over previous
"""Trainium2 Bass kernel for nn_DisentangledAttention (DeBERTa-style).

Strategy (batch-parallel over 8 cores, one batch element per core):
  - Only 2S-1 = 767 distinct relative positions exist, so the position
    projections run on the 767-row table instead of the [S,S,E] gather
    (cuts ~77G MACs to ~0.4G).
  - content-position (bd) and position-content (bc) scores are computed in
    "relative" coordinates [i, r] / [j, r'] by plain matmuls, then moved to
    absolute [i, j] coordinates with a skewed SBUF->SBUF DMA: a diagonal
    flat access pattern (step = row_stride - 1) gives each partition a
    free-dim offset of -1 per partition, which is exactly the relative->
    absolute shear.  bc lands transposed and is added into the score PSUM
    with PE transpose-accumulate matmuls.
  - matmuls run as float32r (fp32 bits, 1 PE cycle/row at N>=256 vs 4 for
    strict fp32).
  - softmax: ACT exp (scale=1/8 folded in) with fused row-sum accumulator,
    DVE reciprocal + per-partition scale.
  - attn @ V consumes w^T produced by PE transposes of the weight tiles;
    the result lands head-transposed [E, S] which feeds the final fc
    matmul directly (lhsT layout), so the fc output is row-major [S, E].
"""

import sys

sys.path.insert(0, "/opt/trn_rl_repo")

import numpy as np

S = 384
E = 512
H = 8
D = 64
P = 128
R = 2 * S - 1          # 767 distinct relative offsets
RP = 768               # padded relative width (f32r needs even N)
BAND = 511             # valid band width per 128-row tile
BW = 512               # allocated band width
NE = E // P            # 4
NS = S // P            # 3
MAX_SEQ = 512
B = 8

_CACHE = {}


def _build(loop_n=0):
    import concourse.bacc as bacc
    import concourse.mybir as mybir
    from concourse.tile import TileContext
    from concourse.masks import make_identity

    f32 = mybir.dt.float32
    f32r = mybir.dt.float32r

    nc = bacc.Bacc("TRN2", num_devices=B, debug=False)

    xT = nc.dram_tensor("xT", [E, S], f32r, kind="ExternalInput")
    qW = nc.dram_tensor("qW", [E, E], f32r, kind="ExternalInput")
    kW = nc.dram_tensor("kW", [E, E], f32r, kind="ExternalInput")
    vW = nc.dram_tensor("vW", [E, E], f32r, kind="ExternalInput")
    qpW = nc.dram_tensor("qpW", [E, E], f32r, kind="ExternalInput")
    kpW = nc.dram_tensor("kpW", [E, E], f32r, kind="ExternalInput")
    fcW = nc.dram_tensor("fcW", [E, E], f32r, kind="ExternalInput")
    posT = nc.dram_tensor("posT", [E, RP], f32r, kind="ExternalInput")
    bias4 = nc.dram_tensor("bias4", [P, 16], f32, kind="ExternalInput")
    bias2 = nc.dram_tensor("bias2", [1, 2 * E + P], f32r, kind="ExternalInput")

    w_out = nc.dram_tensor("w_out", [H, S, S], f32, kind="ExternalOutput")
    out_b = nc.dram_tensor("out_b", [S, E], f32, kind="ExternalOutput")

    Exp = mybir.ActivationFunctionType.Exp
    Copy = mybir.ActivationFunctionType.Copy
    Ident = mybir.ActivationFunctionType.Identity

    def skew_src(tile_ap, nt, length, offset):
        """Diagonal AP: src[p, t, c] = tile[p, t*BW + c + offset - p]."""
        a = tile_ap.copy()
        a.ap = mybir.VecI64Pair([(nt * BW - 1, P), (BW, nt), (1, length)])
        a.offset = offset
        return a

    import contextlib
    with TileContext(nc) as tc:
      with (tc.For_i(0, loop_n, 1) if loop_n else contextlib.nullcontext()):
        with (
            tc.tile_pool(name="const", bufs=1) as cpool,
            tc.tile_pool(name="proj_out", bufs=1) as projp,
            tc.tile_pool(name="pos_out", bufs=1) as posp,
            tc.tile_pool(name="attn_t", bufs=1) as attnp,
        ):
            ident = cpool.tile([P, P], f32)
            make_identity(nc, ident)
            b4 = cpool.tile([P, 16], f32)
            nc.sync.dma_start(b4[:, :], bias4[:, :])
            b2 = cpool.tile([1, 2 * E + P], f32r)
            nc.sync.dma_start(b2[:, :], bias2[:, :])
            ones = b2[0:1, 2 * E : 2 * E + P]

            qT_sb = projp.tile([P, NE * S], f32r)   # [e_out, (m, s)]
            kT_sb = projp.tile([P, NE * S], f32r)
            v_sb = projp.tile([P, NS * E], f32r)    # [s, (m, e)]
            kpT_sb = posp.tile([P, NE * RP], f32r)
            qpTr_sb = posp.tile([P, NE * RP], f32r)  # reversed along r
            attnT_sb = attnp.tile([P, NE * S], f32r)

            # ---------------- phase 1: projections ----------------
            with (
                tc.tile_pool(name="wts", bufs=1) as wpool,
                tc.tile_pool(name="ps_pj", bufs=2, space="PSUM") as pspj,
            ):
                x_sb = wpool.tile([P, NE * S], f32r, tag="x")
                nc.sync.dma_start(
                    x_sb[:, :].rearrange("p (k s) -> p k s", s=S),
                    xT.rearrange("(k p) s -> p k s", p=P),
                )
                posT_sb = wpool.tile([P, NE * RP], f32r, tag="pos")
                nc.sync.dma_start(
                    posT_sb[:, :].rearrange("p (k r) -> p k r", r=RP),
                    posT.rearrange("(k p) r -> p k r", p=P),
                )
                w_tiles = {}
                for name, dram in [("qW", qW), ("kW", kW), ("vW", vW),
                                   ("qpW", qpW), ("kpW", kpW), ("fcW", fcW)]:
                    t = wpool.tile([P, NE * E], f32r, tag=name)
                    nc.sync.dma_start(
                        t[:, :].rearrange("p (k e) -> p k e", e=E),
                        dram.rearrange("(k p) e -> p k e", p=P),
                    )
                    w_tiles[name] = t

                # qT, kT: [e_out, s] with per-partition bias
                for dst, wname, bcol in [(qT_sb, "qW", 0), (kT_sb, "kW", 4)]:
                    wsb = w_tiles[wname]
                    for m in range(NE):
                        ps = pspj.tile([P, S], f32, tag="pj")
                        for k in range(NE):
                            nc.tensor.matmul(
                                ps[:, :],
                                wsb[:, k * E + m * P : k * E + (m + 1) * P],
                                x_sb[:, k * S : (k + 1) * S],
                                start=(k == 0), stop=(k == NE - 1),
                            )
                        nc.vector.tensor_scalar_add(
                            dst[:, m * S : (m + 1) * S], ps[:, :],
                            b4[:, bcol + m : bcol + m + 1],
                        )

                # v: [s, e] natural; bias via K=1 matmul with ones
                for m in range(NS):
                    ps = pspj.tile([P, E], f32, tag="pj")
                    for k in range(NE):
                        nc.tensor.matmul(
                            ps[:, :],
                            x_sb[:, k * S + m * P : k * S + (m + 1) * P],
                            w_tiles["vW"][:, k * E : (k + 1) * E],
                            start=(k == 0), stop=False,
                        )
                    nc.tensor.matmul(
                        ps[:, :], ones, b2[0:1, 0:E],
                        start=False, stop=True, skip_group_check=True,
                    )
                    nc.scalar.activation(v_sb[:, m * E : (m + 1) * E], ps[:, :], Copy)

                # kpT / qpT(reversed): [e_out, r] with per-partition bias
                for dst, wname, bcol, rev in [(kpT_sb, "kpW", 8, False),
                                              (qpTr_sb, "qpW", 12, True)]:
                    wsb = w_tiles[wname]
                    for m in range(NE):
                        for n0, wdt in [(0, 384), (384, 384)]:
                            ps = pspj.tile([P, E], f32, tag="pj")
                            for k in range(NE):
                                nc.tensor.matmul(
                                    ps[:, 0:wdt],
                                    wsb[:, k * E + m * P : k * E + (m + 1) * P],
                                    posT_sb[:, k * RP + n0 : k * RP + n0 + wdt],
                                    start=(k == 0), stop=(k == NE - 1),
                                )
                            if not rev:
                                o = dst[:, m * RP + n0 : m * RP + n0 + wdt]
                            else:
                                o = dst[:, :].copy()
                                o.ap = mybir.VecI64Pair([(NE * RP, P), (-1, wdt)])
                                o.offset = m * RP + (RP - 1) - n0
                            nc.scalar.activation(
                                o, ps[:, 0:wdt], Ident,
                                bias=b4[:, bcol + m : bcol + m + 1],
                            )

            # ---------------- phase 2: attention per head ----------------
            with (
                tc.tile_pool(name="band", bufs=2) as bandp,
                tc.tile_pool(name="skew", bufs=2) as skewp,
                tc.tile_pool(name="soft", bufs=2) as softp,
                tc.tile_pool(name="wT", bufs=2) as wTp,
                tc.tile_pool(name="ps_rel", bufs=2, space="PSUM") as psrel,
                tc.tile_pool(name="ps_s", bufs=2, space="PSUM") as pss,
                tc.tile_pool(name="ps_wt", bufs=3, space="PSUM") as pswt,
                tc.tile_pool(name="ps_o", bufs=1, space="PSUM") as pso,
            ):
                for h in range(H):
                    hb = 64 * (h % 2)
                    mh = h // 2

                    def hsl(tile, base, wdt):
                        return tile[hb : hb + 64, base : base + wdt]

                    # relative-coordinate score matrices (banded, 512-wide)
                    bd_band = bandp.tile([P, NS * BW], f32, tag="bdband", name=f"bdb{h}")
                    bc_band = bandp.tile([P, NS * BW], f32, tag="bcband", name=f"bcb{h}")
                    for t in range(NS):
                        r0 = 256 - P * t
                        psb = psrel.tile([P, BW], f32, tag="rel", name=f"r1{h}{t}")
                        nc.tensor.matmul(
                            psb[:, :],
                            hsl(qT_sb, mh * S + t * P, P),
                            hsl(kpT_sb, mh * RP + r0, BW),
                        )
                        nc.scalar.activation(
                            bd_band[:, t * BW : (t + 1) * BW], psb[:, :], Copy)
                        psb2 = psrel.tile([P, BW], f32, tag="rel", name=f"r2{h}{t}")
                        nc.tensor.matmul(
                            psb2[:, :],
                            hsl(kT_sb, mh * S + t * P, P),
                            hsl(qpTr_sb, mh * RP + r0, BW),
                        )
                        nc.vector.tensor_copy(
                            bc_band[:, t * BW : (t + 1) * BW], psb2[:, :])
                    # relative -> absolute skew (one DMA per matrix)
                    bd_sk = skewp.tile([P, NS * S], f32, tag="bd", name=f"bds{h}")
                    nc.sync.dma_start(
                        bd_sk[:, :].rearrange("p (t c) -> p t c", c=S),
                        skew_src(bd_band[:, :], NS, S, 127),
                    )
                    bcT_sk = skewp.tile([P, NS * S], f32, tag="bcT", name=f"bcs{h}")
                    nc.sync.dma_start(
                        bcT_sk[:, :].rearrange("p (t c) -> p t c", c=S),
                        skew_src(bc_band[:, :], NS, S, 128),
                    )

                    # scores, softmax, weight tiles
                    w_sb = softp.tile([P, NS * S], f32, tag="w", name=f"w{h}")
                    ps_wt_t = [pswt.tile([P, S], f32, tag="wt", name=f"wt{h}_{i}")
                               for i in range(NS)]
                    for t in range(NS):
                        ps_sc = pss.tile([P, S], f32, tag="sc", name=f"sc{h}{t}")
                        nc.tensor.matmul(
                            ps_sc[:, :],
                            hsl(qT_sb, mh * S + t * P, P),
                            hsl(kT_sb, mh * S, S),
                            start=True, stop=False,
                        )
                        for tj in range(NS):
                            nc.tensor.matmul(
                                ps_sc[:, tj * P : (tj + 1) * P],
                                bcT_sk[:, tj * S + t * P : tj * S + (t + 1) * P],
                                ident[:, :],
                                is_transpose=True,
                                start=False, stop=(tj == NS - 1),
                                skip_group_check=True,
                            )
                        s1 = softp.tile([P, S], f32, tag="s1", name=f"s1{h}{t}")
                        nc.vector.tensor_add(
                            s1[:, :], ps_sc[:, :], bd_sk[:, t * S : (t + 1) * S])
                        ssum = softp.tile([P, 1], f32, tag="ssum", name=f"ss{h}{t}")
                        es = softp.tile([P, S], f32, tag="es", name=f"es{h}{t}")
                        nc.scalar.activation(es[:, :], s1[:, :], Exp,
                                             scale=0.125, accum_out=ssum[:, :])
                        srec = softp.tile([P, 1], f32, tag="srec", name=f"sr{h}{t}")
                        nc.vector.reciprocal(srec[:, :], ssum[:, :])
                        nc.gpsimd.tensor_scalar_mul(
                            w_sb[:, t * S : (t + 1) * S], es[:, :], srec[:, :])
                        for tj in range(NS):
                            nc.tensor.matmul(
                                ps_wt_t[tj][:, t * P : (t + 1) * P],
                                w_sb[:, t * S + tj * P : t * S + (tj + 1) * P],
                                ident[:, :],
                                is_transpose=True,
                                start=True, stop=True, skip_group_check=True,
                            )

                    # weights output (straight [i, j] write, one DMA per head)
                    nc.scalar.dma_start(
                        w_out[h].rearrange("(t p) j -> p t j", p=P),
                        w_sb[:, :].rearrange("p (t j) -> p t j", j=S),
                    )

                    # attn @ V -> outT [d, i] accumulated over j tiles
                    ps_out = pso.tile([64, S], f32, tag="av", name=f"av{h}")
                    for tj in range(NS):
                        wT_sb = wTp.tile([P, S], f32r, tag="wT", name=f"wk{h}{tj}")
                        nc.vector.tensor_copy(wT_sb[:, :], ps_wt_t[tj][:, :])
                        nc.tensor.matmul(
                            ps_out[:, :],
                            v_sb[:, tj * E + 64 * h : tj * E + 64 * h + 64],
                            wT_sb[:, :],
                            start=(tj == 0), stop=(tj == NS - 1),
                        )
                    nc.scalar.activation(
                        attnT_sb[hb : hb + 64, mh * S : (mh + 1) * S],
                        ps_out[:, :], Copy)

            # ---------------- phase 3: fc ----------------
            with (
                tc.tile_pool(name="fc_sb", bufs=1) as fcp,
                tc.tile_pool(name="ps_fc", bufs=2, space="PSUM") as psfc,
            ):
                fcW_sb = fcp.tile([P, NE * E], f32r, tag="fcw")
                nc.sync.dma_start(
                    fcW_sb[:, :].rearrange("p (k e) -> p k e", e=E),
                    fcW.rearrange("(k p) e -> p k e", p=P),
                )
                for m in range(NS):
                    ps = psfc.tile([P, E], f32, tag="fc", name=f"fc{m}")
                    for k in range(NE):
                        nc.tensor.matmul(
                            ps[:, :],
                            attnT_sb[:, k * S + m * P : k * S + (m + 1) * P],
                            fcW_sb[:, k * E : (k + 1) * E],
                            start=(k == 0), stop=False,
                        )
                    nc.tensor.matmul(
                        ps[:, :], ones, b2[0:1, E : 2 * E],
                        start=False, stop=True, skip_group_check=True,
                    )
                    o_sb = fcp.tile([P, E], f32, tag="o", name=f"o{m}")
                    nc.scalar.activation(o_sb[:, :], ps[:, :], Copy)
                    nc.scalar.dma_start(out_b[m * P : (m + 1) * P, :], o_sb[:, :])

    nc.compile()
    return nc


def kernel(x, qW, qb, kW, kb, vW, vb, qpW, qpb, kpW, kpb, pos_table, fcW, fcb):
    from concourse import bass_utils

    if "nc" not in _CACHE:
        _CACHE["nc"] = _build()
    nc = _CACHE["nc"]

    f = lambda a: np.ascontiguousarray(np.asarray(a), dtype=np.float32)
    x, pos_table = f(x), f(pos_table)
    posT = np.zeros((E, RP), np.float32)
    posT[:, :R] = pos_table[MAX_SEQ - S : MAX_SEQ - S + R].T
    bias4 = np.concatenate(
        [f(b).reshape(NE, P).T for b in (qb, kb, kpb, qpb)], axis=1)
    bias2 = np.concatenate([f(vb), f(fcb), np.ones(P, np.float32)]).reshape(1, 2 * E + P)
    base = {"qW": f(qW), "kW": f(kW), "vW": f(vW), "qpW": f(qpW),
            "kpW": f(kpW), "fcW": f(fcW), "posT": np.ascontiguousarray(posT),
            "bias4": np.ascontiguousarray(bias4), "bias2": np.ascontiguousarray(bias2)}
    in_maps = [dict(base, xT=np.ascontiguousarray(x[b].T)) for b in range(B)]

    res = bass_utils.run_bass_kernel_spmd(nc, in_maps, core_ids=list(range(B)))
    out = np.stack([r["out_b"] for r in res.results])
    weights = np.stack([r["w_out"] for r in res.results])
    _CACHE["last_results"] = res
    return out, weights
